# revision 11
# baseline (speedup 1.0000x reference)
"""ChannelKiller kernel for Trainium2 (8 NeuronCores, SPMD).

Computes out[b, c, t] = x[b, c, t] * (1.0 if c == 0 else 0.5) for
x of shape (16, 8, 262144) f32.

Memory-bound elementwise op; per-core HBM roofline is ~94 us (16 MiB in +
16 MiB out at ~358 GB/s). Sharding: batch-parallel, core i gets x[2i:2i+2];
no communication. Each per-core batch (8, 262144) is viewed as
[128 partitions x 16384] so channel == partition//16 and the scale becomes a
per-partition [128,1] vector (1.0 on partitions 0..15, 0.5 elsewhere)
supplied as a second input.

The kernel is hand-scheduled raw bacc (no Tile framework) because Tile's
kernel-exit drain + all-engine EVSEM barrier costs ~20 us per invocation on
HW; measured one-shot here is ~101 us vs ~123 us for the equivalent Tile
version. Structure: 10 SBUF slots of [128, 4096] f32;

  SP (sync)   : [wait slot's store done] DMA load slot -> inc ld[s]
  DVE (vector): wait ld[s] -> tensor_scalar_mul by scale vec -> inc mul
  ACT (scalar): wait mul >= k+1 -> DMA store slot -> inc st[s]

ld[s]/st[s] are per-slot DMA semaphores so wait thresholds stay exact under
any cross-queue DMA completion order; the kernel ends with SP waiting on all
store semaphores (completion guarantee) instead of a 5-engine barrier.
Verified bit-exact vs the reference (CoreSim race detector + hardware).
"""

import numpy as np

import concourse.bacc as bacc
import concourse.mybir as mybir
from concourse.bass_utils import run_bass_kernel_spmd

N_CORES = 8
B, C, T = 16, 8, 262144
B_LOC = B // N_CORES            # batches per core = 2
P = 128                         # SBUF partitions
ROWS_PER_BATCH = C * T // P     # free elems per partition per batch = 16384
P_PER_C = P // C                # partitions per channel = 16
TILE_F = 4096                   # free-dim tile size (16 KiB/partition, 2 MiB/tile)
BUFS = 10

_NC_CACHE = None


def _build():
    global _NC_CACHE
    if _NC_CACHE is not None:
        return _NC_CACHE
    n_pb = ROWS_PER_BATCH // TILE_F          # tiles per batch
    n = B_LOC * n_pb                         # tiles per core
    nc = bacc.Bacc("TRN2", target_bir_lowering=False, debug=False, num_devices=N_CORES)
    x = nc.declare_dram_parameter(
        "x", [B_LOC, P, ROWS_PER_BATCH], mybir.dt.float32, isOutput=False
    )
    scale_in = nc.declare_dram_parameter(
        "scale", [P, 1], mybir.dt.float32, isOutput=False
    )
    out = nc.declare_dram_parameter(
        "out", [B_LOC, P, ROWS_PER_BATCH], mybir.dt.float32, isOutput=True
    )

    def src(k):
        b, t = divmod(k, n_pb)
        return x[b][:, t * TILE_F : (t + 1) * TILE_F]

    def dst(k):
        b, t = divmod(k, n_pb)
        return out[b][:, t * TILE_F : (t + 1) * TILE_F]

    with (
        nc.sbuf_tensor([P, BUFS * TILE_F], mybir.dt.float32) as buf,
        nc.sbuf_tensor([P, 1], mybir.dt.float32) as scale,
        nc.Block() as block,
    ):
        ld = [nc.semaphore(f"ld{s}").__enter__() for s in range(BUFS)]
        st = [nc.semaphore(f"st{s}").__enter__() for s in range(BUFS)]
        mul_sem = nc.semaphore("mul").__enter__()
        sc_sem = nc.semaphore("sc").__enter__()

        def tile(s):
            return buf[:, s * TILE_F : (s + 1) * TILE_F]

        @block.sync
        def _(sync):
            sync.dma_start(scale[:, :], scale_in[:, :]).then_inc(sc_sem, 16)
            for k in range(n):
                s = k % BUFS
                if k >= BUFS:
                    sync.wait_ge(st[s], 16 * (k // BUFS))
                sync.dma_start(tile(s), src(k)).then_inc(ld[s], 16)
            for s in range(BUFS):
                total = 16 * len([k for k in range(n) if k % BUFS == s])
                if total:
                    sync.wait_ge(st[s], total)

        @block.vector
        def _(vector):
            vector.wait_ge(sc_sem, 16)
            for k in range(n):
                s = k % BUFS
                vector.wait_ge(ld[s], 16 * (k // BUFS + 1))
                nc.vector.tensor_scalar_mul(tile(s), tile(s), scale[:, 0:1]).then_inc(
                    mul_sem, 1
                )

        @block.scalar
        def _(scalar):
            for k in range(n):
                s = k % BUFS
                scalar.wait_ge(mul_sem, k + 1)
                scalar.dma_start(dst(k), tile(s)).then_inc(st[s], 16)

    nc.finalize()
    _NC_CACHE = nc
    return nc


def kernel(x: np.ndarray) -> np.ndarray:
    x = np.ascontiguousarray(np.asarray(x, dtype=np.float32))
    assert x.shape == (B, C, T), x.shape
    nc = _build()

    scale_np = np.full((P, 1), 0.5, dtype=np.float32)
    scale_np[:P_PER_C] = 1.0  # partitions 0..15 hold channel 0

    shards = x.reshape(N_CORES, B_LOC, P, ROWS_PER_BATCH)
    in_maps = [{"x": shards[i], "scale": scale_np} for i in range(N_CORES)]
    r = run_bass_kernel_spmd(nc, in_maps, list(range(N_CORES)))

    out = np.concatenate(
        [r.results[i]["out"].reshape(B_LOC, C, T) for i in range(N_CORES)], axis=0
    )
    return out


# revision 14
# speedup vs baseline: 1.0062x; 1.0062x over previous
"""ChannelKiller kernel for Trainium2 (8 NeuronCores, SPMD).

Computes out[b, c, t] = x[b, c, t] * (1.0 if c == 0 else 0.5) for
x of shape (16, 8, 262144) f32.

Memory-bound elementwise op; per-core HBM roofline is ~94 us (16 MiB in +
16 MiB out at ~358 GB/s). Sharding: batch-parallel, core i gets x[2i:2i+2];
no communication. Each per-core batch (8, 262144) is viewed as
[128 partitions x 16384] so channel == partition//16 and the scale becomes a
per-partition [128,1] vector (1.0 on partitions 0..15, 0.5 elsewhere)
supplied as a second input.

The kernel is hand-scheduled raw bacc (no Tile framework) because Tile's
kernel-exit drain + all-engine EVSEM barrier costs ~20 us per invocation on
HW; measured one-shot here is ~101 us vs ~123 us for the equivalent Tile
version. Structure: 10 SBUF slots of [128, 4096] f32;

  SP (sync)    : even-k loads via HWDGE queue  -> inc ld[s]
  GpSimd       : odd-k loads via SWDGE queue   -> inc ld[s]
  DVE (vector) : wait ld[s] -> tensor_scalar_mul by scale vec -> inc mul
  ACT (scalar) : wait mul >= k+1 -> DMA store slot -> inc st[s]

Loads alternate between the two independent DMA descriptor paths (SP/HWDGE
and GpSimd/SWDGE) so two hardware queues generate and process load
descriptors in parallel (measured ~2 us better and tighter variance than
single-queue loads). ld[s]/st[s] are per-slot DMA semaphores so wait
thresholds stay exact under any cross-queue DMA completion order; the kernel
ends with SP waiting on all store semaphores (completion guarantee) instead
of a 5-engine barrier. Verified bit-exact vs the reference (CoreSim race
detector + hardware).
"""

import numpy as np

import concourse.bacc as bacc
import concourse.mybir as mybir
from concourse.bass_utils import run_bass_kernel_spmd

N_CORES = 8
B, C, T = 16, 8, 262144
B_LOC = B // N_CORES            # batches per core = 2
P = 128                         # SBUF partitions
ROWS_PER_BATCH = C * T // P     # free elems per partition per batch = 16384
P_PER_C = P // C                # partitions per channel = 16
TILE_F = 4096                   # free-dim tile size (16 KiB/partition, 2 MiB/tile)
BUFS = 10

_NC_CACHE = None


def _build():
    global _NC_CACHE
    if _NC_CACHE is not None:
        return _NC_CACHE
    n_pb = ROWS_PER_BATCH // TILE_F          # tiles per batch
    n = B_LOC * n_pb                         # tiles per core
    nc = bacc.Bacc("TRN2", target_bir_lowering=False, debug=False, num_devices=N_CORES)
    x = nc.declare_dram_parameter(
        "x", [B_LOC, P, ROWS_PER_BATCH], mybir.dt.float32, isOutput=False
    )
    scale_in = nc.declare_dram_parameter(
        "scale", [P, 1], mybir.dt.float32, isOutput=False
    )
    out = nc.declare_dram_parameter(
        "out", [B_LOC, P, ROWS_PER_BATCH], mybir.dt.float32, isOutput=True
    )

    def src(k):
        b, t = divmod(k, n_pb)
        return x[b][:, t * TILE_F : (t + 1) * TILE_F]

    def dst(k):
        b, t = divmod(k, n_pb)
        return out[b][:, t * TILE_F : (t + 1) * TILE_F]

    with (
        nc.sbuf_tensor([P, BUFS * TILE_F], mybir.dt.float32) as buf,
        nc.sbuf_tensor([P, 1], mybir.dt.float32) as scale,
        nc.Block() as block,
    ):
        ld = [nc.semaphore(f"ld{s}").__enter__() for s in range(BUFS)]
        st = [nc.semaphore(f"st{s}").__enter__() for s in range(BUFS)]
        mul_sem = nc.semaphore("mul").__enter__()
        sc_sem = nc.semaphore("sc").__enter__()

        def tile(s):
            return buf[:, s * TILE_F : (s + 1) * TILE_F]

        def load_stream(eng, parity):
            for k in range(n):
                if k % 2 != parity:
                    continue
                s = k % BUFS
                if k >= BUFS:
                    eng.wait_ge(st[s], 16 * (k // BUFS))
                eng.dma_start(tile(s), src(k)).then_inc(ld[s], 16)

        @block.sync
        def _(sync):
            load_stream(sync, 0)
            for s in range(BUFS):
                total = 16 * len([k for k in range(n) if k % BUFS == s])
                if total:
                    sync.wait_ge(st[s], total)

        @block.gpsimd
        def _(gpsimd):
            load_stream(gpsimd, 1)

        @block.vector
        def _(vector):
            vector.wait_ge(sc_sem, 16)
            for k in range(n):
                s = k % BUFS
                vector.wait_ge(ld[s], 16 * (k // BUFS + 1))
                nc.vector.tensor_scalar_mul(tile(s), tile(s), scale[:, 0:1]).then_inc(
                    mul_sem, 1
                )

        @block.scalar
        def _(scalar):
            scalar.dma_start(scale[:, :], scale_in[:, :]).then_inc(sc_sem, 16)
            for k in range(n):
                s = k % BUFS
                scalar.wait_ge(mul_sem, k + 1)
                scalar.dma_start(dst(k), tile(s)).then_inc(st[s], 16)

    nc.finalize()
    _NC_CACHE = nc
    return nc


def kernel(x: np.ndarray) -> np.ndarray:
    x = np.ascontiguousarray(np.asarray(x, dtype=np.float32))
    assert x.shape == (B, C, T), x.shape
    nc = _build()

    scale_np = np.full((P, 1), 0.5, dtype=np.float32)
    scale_np[:P_PER_C] = 1.0  # partitions 0..15 hold channel 0

    shards = x.reshape(N_CORES, B_LOC, P, ROWS_PER_BATCH)
    in_maps = [{"x": shards[i], "scale": scale_np} for i in range(N_CORES)]
    r = run_bass_kernel_spmd(nc, in_maps, list(range(N_CORES)))

    out = np.concatenate(
        [r.results[i]["out"].reshape(B_LOC, C, T) for i in range(N_CORES)], axis=0
    )
    return out
